# revision 2
# baseline (speedup 1.0000x reference)
"""Raw-bass (manual semaphore) version of the DigitConvolutionalModel kernel.

Per core: 2048 samples, 4 batch tiles of NB=512. All activations live
transposed (features on partitions, batch on free dim). Matmuls in fp32r.

Engine roles:
  sync   - x even chunks DMA (HWDGE Q1) + out DMA triggers
  scalar - weight DMAs + x odd chunks DMA (HWDGE Q10) + L1 relu (x2)
  vector - L2/L3 relu+bias, L4 bias add
  tensor - warmup + all matmuls
  gpsimd - warm tile memset only

Semaphore design constraints (race-checker-sound): DMA completions on one
queue are treated as unordered, so every DMA wait is an "all producers
done" threshold on a dedicated semaphore. Compute sems are inc-by-1 from a
single engine, so intermediate thresholds are fine.
"""

from contextlib import ExitStack

import numpy as np

import concourse.bass as bass
import concourse.mybir as mybir

N_CORES = 8
B = 16384
BC = B // N_CORES
NB = 512
NT = BC // NB  # 4
KC = 112
NKC = 7  # K chunks of layer 1

F32 = mybir.dt.float32
F32R = mybir.dt.float32r
RELU = mybir.ActivationFunctionType.Relu
ADD = mybir.AluOpType.add
MAX = mybir.AluOpType.max

N_WARM_MM = 10

# x chunk -> queue: even chunks on sync, odd on scalar
SYNC_CHUNKS = [0, 2, 4, 6]
SCAL_CHUNKS = [1, 3, 5]
XBUF = 3  # x tile sets (triple buffer)


def build_program():
    nc = bass.Bass()

    xt_d = nc.declare_dram_parameter("xt", [NT, KC, NKC * NB], F32R, isOutput=False)
    w1_d = nc.declare_dram_parameter("w1e", [KC, NKC * 256], F32R, isOutput=False)
    w2_d = nc.declare_dram_parameter("w2t", [128, 2 * 128], F32R, isOutput=False)
    w3_d = nc.declare_dram_parameter("w3t", [128, 64], F32R, isOutput=False)
    w4_d = nc.declare_dram_parameter("w4t", [64, 10], F32R, isOutput=False)
    b1_d = nc.declare_dram_parameter("b1", [128, 2], F32, isOutput=False)
    b2_d = nc.declare_dram_parameter("b2", [128, 1], F32, isOutput=False)
    b3_d = nc.declare_dram_parameter("b3", [64, 1], F32, isOutput=False)
    b4_d = nc.declare_dram_parameter("b4", [10, 1], F32, isOutput=False)
    out_d = nc.declare_dram_parameter("outT", [10, BC], F32, isOutput=True)

    ctx = ExitStack()
    with ctx:
        # SBUF
        xsb = ctx.enter_context(nc.sbuf_tensor([KC, XBUF, NKC, NB], F32R))
        w1sb = ctx.enter_context(nc.sbuf_tensor([KC, NKC, 256], F32R))
        w2sb = ctx.enter_context(nc.sbuf_tensor([128, 2, 128], F32R))
        w3sb = ctx.enter_context(nc.sbuf_tensor([128, 64], F32R))
        w4sb = ctx.enter_context(nc.sbuf_tensor([64, 10], F32R))
        b1sb = ctx.enter_context(nc.sbuf_tensor([128, 2], F32))
        b2sb = ctx.enter_context(nc.sbuf_tensor([128, 1], F32))
        b3sb = ctx.enter_context(nc.sbuf_tensor([64, 1], F32))
        b4sb = ctx.enter_context(nc.sbuf_tensor([10, 1], F32))
        h1sb = ctx.enter_context(nc.sbuf_tensor([128, 2, 2, NB], F32R))
        h2sb = ctx.enter_context(nc.sbuf_tensor([128, 2, NB], F32R))
        h3sb = ctx.enter_context(nc.sbuf_tensor([64, 2, NB], F32R))
        osb = ctx.enter_context(nc.sbuf_tensor([10, 2, NB], F32))
        warm_w = ctx.enter_context(nc.sbuf_tensor([64, 16], mybir.dt.bfloat16))
        warm_x = ctx.enter_context(nc.sbuf_tensor([64, NB], mybir.dt.bfloat16))

        # PSUM: 4 + 1 + 1 + 1 + 1 = 8 banks
        ps1 = ctx.enter_context(nc.psum_tensor([128, 2, 2, NB], F32))
        ps2 = ctx.enter_context(nc.psum_tensor([128, NB], F32))
        ps3 = ctx.enter_context(nc.psum_tensor([64, NB], F32))
        ps4 = ctx.enter_context(nc.psum_tensor([10, NB], F32))
        psw = ctx.enter_context(nc.psum_tensor([16, NB], F32))

        # one-shot chunk sems for tile 0: x chunk + w1 chunk both inc -> wait 32
        sc = [ctx.enter_context(nc.semaphore(f"sc{c}")) for c in range(NKC)]
        # per-tile batch sems for tiles 1..3
        sxa = [None] + [
            ctx.enter_context(nc.semaphore(f"sxa{t}")) for t in range(1, NT)
        ]
        sxb = [None] + [
            ctx.enter_context(nc.semaphore(f"sxb{t}")) for t in range(1, NT)
        ]
        swr = ctx.enter_context(nc.semaphore("swr"))  # 7 remaining weight DMAs
        sm = ctx.enter_context(nc.semaphore("sm"))  # PE: L1 psum groups done
        s2 = ctx.enter_context(nc.semaphore("s2"))  # PE: L2/L3/L4 done
        sa = ctx.enter_context(nc.semaphore("sa"))  # ACT: h1 relus done
        sv = ctx.enter_context(nc.semaphore("sv"))  # DVE: h2/h3/out done
        so = [ctx.enter_context(nc.semaphore(f"so{t}")) for t in range(NT)]
        sg = ctx.enter_context(nc.semaphore("sg"))  # warm memset done

        block = ctx.enter_context(nc.Block())

        def x_dma(eng, t, c, sem):
            eng.dma_start(
                out=xsb[:, t % XBUF, c, :],
                in_=xt_d[t, :, c * NB : (c + 1) * NB],
            ).then_inc(sem, 16)

        @block.gpsimd
        def _(gp):
            gp.memset(warm_w[:], 0.125).then_inc(sg, 1)
            gp.memset(warm_x[:], 0.125).then_inc(sg, 1)

        @block.sync
        def _(sy):
            # tile 0 even chunks feed the one-shot chunk sems
            for c in SYNC_CHUNKS:
                x_dma(sy, 0, c, sc[c])
            for t in (1, 2):
                for c in SYNC_CHUNKS:
                    x_dma(sy, t, c, sxa[t])
            for t in range(NT):
                if t + 3 < NT:
                    sy.wait_ge(sm, 2 * t + 2)  # L1(t) done -> set t%XBUF free
                    for c in SYNC_CHUNKS:
                        x_dma(sy, t + 3, c, sxa[t + 3])
                sy.wait_ge(sv, 3 * t + 3)  # out(t) in osb
                sy.dma_start(
                    out=out_d[:, t * NB : (t + 1) * NB],
                    in_=osb[:, t % 2, :],
                ).then_inc(so[t], 16)
            for t in range(NT):
                sy.wait_ge(so[t], 16)

        @block.scalar
        def _(sc_eng):
            # w1 chunks + tile-0 odd x chunks, interleaved by need order
            def w1_dma(c):
                sc_eng.dma_start(
                    out=w1sb[:, c, :], in_=w1_d[:, c * 256 : (c + 1) * 256]
                ).then_inc(sc[c], 16)

            w1_dma(0)
            w1_dma(1)
            x_dma(sc_eng, 0, 1, sc[1])
            w1_dma(2)
            w1_dma(3)
            x_dma(sc_eng, 0, 3, sc[3])
            w1_dma(4)
            w1_dma(5)
            x_dma(sc_eng, 0, 5, sc[5])
            w1_dma(6)
            sc_eng.dma_start(out=b1sb[:], in_=b1_d[:]).then_inc(swr, 16)
            sc_eng.dma_start(
                out=w2sb[:], in_=w2_d[:].rearrange("p (c o) -> p c o", c=2)
            ).then_inc(swr, 16)
            sc_eng.dma_start(out=b2sb[:], in_=b2_d[:]).then_inc(swr, 16)
            sc_eng.dma_start(out=w3sb[:], in_=w3_d[:]).then_inc(swr, 16)
            sc_eng.dma_start(out=b3sb[:], in_=b3_d[:]).then_inc(swr, 16)
            sc_eng.dma_start(out=w4sb[:], in_=w4_d[:]).then_inc(swr, 16)
            sc_eng.dma_start(out=b4sb[:], in_=b4_d[:]).then_inc(swr, 16)
            for t in (1, 2):
                for c in SCAL_CHUNKS:
                    x_dma(sc_eng, t, c, sxb[t])
            sc_eng.wait_ge(swr, 112)  # b1 present (all-done threshold)
            for t in range(NT):
                st = t % 2
                if t >= 2:
                    sc_eng.wait_ge(s2, 3 * (t - 2) + 1)  # PE done reading h1 set
                sc_eng.wait_ge(sm, 2 * t + 1)
                sc_eng.activation(
                    h1sb[:, st, 0, :], ps1[:, st, 0, :], RELU, bias=b1sb[:, 0:1]
                ).then_inc(sa, 1)
                sc_eng.wait_ge(sm, 2 * t + 2)
                sc_eng.activation(
                    h1sb[:, st, 1, :], ps1[:, st, 1, :], RELU, bias=b1sb[:, 1:2]
                ).then_inc(sa, 1)
                if t + 3 < NT:
                    # sm >= 2t+2 observed -> x set t%XBUF free for rewrite
                    for c in SCAL_CHUNKS:
                        x_dma(sc_eng, t + 3, c, sxb[t + 3])

        @block.vector
        def _(ve):
            ve.wait_ge(swr, 112)
            for t in range(NT):
                st = t % 2
                ve.wait_ge(s2, 3 * t + 1)
                ve.tensor_scalar(
                    h2sb[:, st, :], ps2[:], b2sb[:], 0.0, ADD, MAX
                ).then_inc(sv, 1)
                ve.wait_ge(s2, 3 * t + 2)
                ve.tensor_scalar(
                    h3sb[:, st, :], ps3[:], b3sb[:], 0.0, ADD, MAX
                ).then_inc(sv, 1)
                ve.wait_ge(s2, 3 * t + 3)
                if t >= 2:
                    ve.wait_ge(so[t - 2], 16)  # osb set free (out DMA done)
                ve.tensor_scalar(
                    osb[:, st, :], ps4[:], b4sb[:], None, ADD
                ).then_inc(sv, 1)

        @block.tensor
        def _(te):
            # warm up the PE clock while DMAs land
            te.wait_ge(sg, 2)
            for _i in range(N_WARM_MM):
                te.matmul(psw[:], warm_w[:], warm_x[:], start=True, stop=True)
            for t in range(NT):
                st = t % 2
                if t >= 2:
                    te.wait_ge(sa, 2 * (t - 2) + 2)  # ps1 set free (ACT drained)
                if t >= 1:
                    te.wait_ge(sxa[t], 16 * len(SYNC_CHUNKS))
                    te.wait_ge(sxb[t], 16 * len(SCAL_CHUNKS))
                for c in range(NKC):
                    if t == 0:
                        te.wait_ge(sc[c], 32)  # x chunk + w1 chunk both landed
                    for m in range(2):
                        mm = te.matmul(
                            ps1[:, st, m, :],
                            w1sb[:, c, m * 128 : (m + 1) * 128],
                            xsb[:, t % XBUF, c, :],
                            start=(c == 0),
                            stop=(c == NKC - 1),
                        )
                        if c == NKC - 1:
                            mm.then_inc(sm, 1)
                # L2
                if t == 0:
                    te.wait_ge(swr, 112)
                te.wait_ge(sa, 2 * t + 2)
                te.matmul(
                    ps2[:], w2sb[:, 0, :], h1sb[:, st, 0, :], start=True, stop=False
                )
                te.matmul(
                    ps2[:], w2sb[:, 1, :], h1sb[:, st, 1, :], start=False, stop=True
                ).then_inc(s2, 1)
                # L3
                te.wait_ge(sv, 3 * t + 1)
                te.matmul(
                    ps3[:], w3sb[:], h2sb[:, st, :], start=True, stop=True
                ).then_inc(s2, 1)
                # L4
                te.wait_ge(sv, 3 * t + 2)
                te.matmul(
                    ps4[:], w4sb[:], h3sb[:, st, :], start=True, stop=True
                ).then_inc(s2, 1)

    return nc


def prepare_inputs(x, conv_w, w1, b1, w2, b2, w3, b3, w4, b4):
    """Host-side prep: fold conv into w1, pre-tile/transpose, shard."""
    w1v = np.ascontiguousarray(w1.T).reshape(26, 26, 256)
    w1e = np.zeros((28, 28, 256), dtype=np.float32)
    for di in range(3):
        for dj in range(3):
            w1e[di : di + 26, dj : dj + 26, :] += conv_w[di, dj] * w1v
    w1e = w1e.reshape(784, 256)
    # [KC, NKC*256] with chunk c at cols [c*256, (c+1)*256)
    w1t = np.ascontiguousarray(
        w1e.reshape(NKC, KC, 256).transpose(1, 0, 2)
    ).reshape(KC, NKC * 256)
    w2t = np.ascontiguousarray(
        np.ascontiguousarray(w2.T).reshape(2, 128, 128).transpose(1, 0, 2)
    ).reshape(128, 2 * 128)
    shared = {
        "w1e": w1t,
        "w2t": w2t,
        "w3t": np.ascontiguousarray(w3.T),
        "w4t": np.ascontiguousarray(w4.T),
        "b1": np.ascontiguousarray(b1.reshape(2, 128).T),
        "b2": b2.reshape(128, 1).astype(np.float32),
        "b3": b3.reshape(64, 1).astype(np.float32),
        "b4": b4.reshape(10, 1).astype(np.float32),
    }
    in_maps = []
    for m in range(N_CORES):
        xc = x[m * BC : (m + 1) * BC]  # [2048, 784]
        xt = np.ascontiguousarray(
            xc.reshape(NT, NB, NKC, KC).transpose(0, 3, 2, 1)
        ).reshape(NT, KC, NKC * NB)
        in_maps.append({"xt": xt, **shared})
    return in_maps



_PROGRAM = None


def _get_program():
    global _PROGRAM
    if _PROGRAM is None:
        _PROGRAM = build_program()
    return _PROGRAM


def kernel(x, conv_w, w1, b1, w2, b2, w3, b3, w4, b4):
    from concourse import bass_utils

    args = [x, conv_w, w1, b1, w2, b2, w3, b3, w4, b4]
    x, conv_w, w1, b1, w2, b2, w3, b3, w4, b4 = [
        np.asarray(a, dtype=np.float32) for a in args
    ]
    nc = _get_program()
    in_maps = prepare_inputs(x, conv_w, w1, b1, w2, b2, w3, b3, w4, b4)
    res = bass_utils.run_bass_kernel_spmd(nc, in_maps, list(range(N_CORES)))
    out = np.concatenate(
        [np.ascontiguousarray(res.results[m]["outT"].T) for m in range(N_CORES)],
        axis=0,
    )
    return out.astype(np.float32)


# revision 3
# speedup vs baseline: 1.7360x; 1.7360x over previous
"""Trainium2 Bass kernel for nn_DigitConvolutionalModel (dense CNN -> MLP).

Pure data parallel over 8 NeuronCores (2048 samples each). The 3x3 conv is
linear, so the host folds it into the first FC layer (W1e = C @ w1.T), making
the whole network a 4-layer MLP computed in transposed orientation (features
on partitions, batch on the free dim) in fp16 (psum fp32, ~5e-4 rel err):

    outT = w4t.T @ relu(w3t.T @ relu(w2t.T @ relu(W1e.T @ xT + b1) + b2) + b3) + b4

Raw bass with manual semaphores. Single-queue need-ordered DMAs; the PE op
order software-pipelines tiles; L3-relu runs on ACT, L2-relu/out-bias on DVE.

PE op order (A=L1, B=L2, C=L3, D=L4):
  A0 A1 B0 A2 C0 B1 A3 D0 C1 B2 D1 C2 B3 D2 C3 D3
ACT: r00 r01 r10 r11 r20 r21 h3(0) r30 r31 h3(1) h3(2) h3(3)   (sa +1 each)
DVE: h2(0) h2(1) out(0) h2(2) out(1) h2(3) out(2) out(3)       (sv +1 each)
s2 counts PE tail ops (B/C/D) in PE order.
"""

from contextlib import ExitStack

import ml_dtypes
import numpy as np

import concourse.bass as bass
import concourse.mybir as mybir

N_CORES = 8
B = 16384
BC = B // N_CORES
NB = 512
NT = BC // NB
KC = 112
NKC = 7

F32 = mybir.dt.float32
BF16 = mybir.dt.bfloat16
FP16 = mybir.dt.float16
RELU = mybir.ActivationFunctionType.Relu
ADD = mybir.AluOpType.add
MAX = mybir.AluOpType.max

N_WARM_MM = 5

X_SPLITS = [
    [(0, 1), (1, 2), (2, 4), (4, 7)],
    [(0, 3), (3, 7)],
    [(0, 3), (3, 7)],
    [(0, 3), (3, 7)],
]
W1_SPLITS = [(0, 3), (3, 7)]

PE_ORDER = [
    ("A", 0), ("A", 1), ("B", 0), ("A", 2), ("C", 0), ("B", 1), ("A", 3),
    ("D", 0), ("C", 1), ("B", 2), ("D", 1), ("C", 2), ("B", 3), ("D", 2),
    ("C", 3), ("D", 3),
]
TAILS = [(k, t) for (k, t) in PE_ORDER if k != "A"]
POS_PE = {op: i + 1 for i, op in enumerate(TAILS)}  # s2 thresholds

ACT_ORDER = [
    ("r", 0, 0), ("r", 0, 1), ("r", 1, 0), ("r", 1, 1), ("r", 2, 0),
    ("r", 2, 1), ("h3", 0), ("r", 3, 0), ("r", 3, 1), ("h3", 1),
    ("h3", 2), ("h3", 3),
]
POS_A = {op: i + 1 for i, op in enumerate(ACT_ORDER)}  # sa thresholds

DVE_ORDER = [
    ("h2", 0), ("h2", 1), ("out", 0), ("h2", 2), ("out", 1), ("h2", 3),
    ("out", 2), ("out", 3),
]
POS_V = {op: i + 1 for i, op in enumerate(DVE_ORDER)}  # sv thresholds


def build_program(l1_dt=FP16, l234_dt=FP16):
    nc = bass.Bass()

    n_wp = 256 + 64 + 10

    xt_d = nc.declare_dram_parameter("xt", [NT, KC, NKC * NB], l1_dt, isOutput=False)
    w1_d = nc.declare_dram_parameter("w1e", [KC, NKC * 256], l1_dt, isOutput=False)
    wp_d = nc.declare_dram_parameter("wpack", [128, n_wp], l234_dt, isOutput=False)
    bp_d = nc.declare_dram_parameter("bpack", [128, 5], F32, isOutput=False)
    out_d = nc.declare_dram_parameter("outT", [10, BC], F32, isOutput=True)

    ctx = ExitStack()
    with ctx:
        xsb = ctx.enter_context(nc.sbuf_tensor([KC, NT, NKC, NB], l1_dt))
        w1sb = ctx.enter_context(nc.sbuf_tensor([KC, NKC, 256], l1_dt))
        wpsb = ctx.enter_context(nc.sbuf_tensor([128, n_wp], l234_dt))
        bpsb = ctx.enter_context(nc.sbuf_tensor([128, 5], F32))
        h1sb = ctx.enter_context(nc.sbuf_tensor([128, 2, 2, NB], l234_dt))
        h2sb = ctx.enter_context(nc.sbuf_tensor([128, 2, NB], l234_dt))
        h3sb = ctx.enter_context(nc.sbuf_tensor([64, 2, NB], l234_dt))
        osb = ctx.enter_context(nc.sbuf_tensor([10, NT, NB], F32))
        warm = ctx.enter_context(nc.sbuf_tensor([1, 513], BF16))
        dump_a = ctx.enter_context(nc.sbuf_tensor([1, 16], BF16))
        dump_v = ctx.enter_context(nc.sbuf_tensor([1, 16], BF16))

        w2v = wpsb[:, 0:256].rearrange("p (c o) -> p c o", c=2)
        w3v = wpsb[:, 256:320]
        w4v = wpsb[0:64, 320:330]
        b1v = bpsb[:, 0:2]
        b2v = bpsb[:, 2:3]
        b3v = bpsb[0:64, 3:4]
        b4v = bpsb[0:10, 4:5]

        ps1 = ctx.enter_context(nc.psum_tensor([128, 2, 2, NB], F32))
        ps2 = ctx.enter_context(nc.psum_tensor([128, NB], F32))
        ps3 = ctx.enter_context(nc.psum_tensor([64, NB], F32))
        ps4 = ctx.enter_context(nc.psum_tensor([10, NB], F32))

        sx = [
            [ctx.enter_context(nc.semaphore(f"sx{t}_{i}")) for i in range(len(X_SPLITS[t]))]
            for t in range(NT)
        ]
        sw1 = [ctx.enter_context(nc.semaphore(f"sw1_{i}")) for i in range(len(W1_SPLITS))]
        swr = ctx.enter_context(nc.semaphore("swr"))
        sm = ctx.enter_context(nc.semaphore("sm"))
        s2 = ctx.enter_context(nc.semaphore("s2"))
        sa = ctx.enter_context(nc.semaphore("sa"))
        sv = ctx.enter_context(nc.semaphore("sv"))
        sof = ctx.enter_context(nc.semaphore("sof"))
        sg = ctx.enter_context(nc.semaphore("sg"))

        block = ctx.enter_context(nc.Block())

        @block.sync
        def _(sy):
            # single queue, need-ordered
            def xd(t, i):
                c0, c1 = X_SPLITS[t][i]
                sy.dma_start(
                    out=xsb[:, t, c0:c1, :], in_=xt_d[t, :, c0 * NB : c1 * NB]
                ).then_inc(sx[t][i], 16)

            sy.dma_start(out=w1sb[:, 0:3, :], in_=w1_d[:, 0:768]).then_inc(sw1[0], 16)
            xd(0, 0)
            xd(0, 1)
            sy.dma_start(
                out=w1sb[:, 3:NKC, :], in_=w1_d[:, 768 : NKC * 256]
            ).then_inc(sw1[1], 16)
            xd(0, 2)
            xd(0, 3)
            sy.dma_start(out=wpsb[:], in_=wp_d[:]).then_inc(swr, 16)
            sy.dma_start(out=bpsb[:], in_=bp_d[:]).then_inc(swr, 16)
            for t in range(1, NT):
                for i in range(len(X_SPLITS[t])):
                    xd(t, i)
            for t in range(NT):
                sy.wait_ge(sv, POS_V[("out", t)])
                sy.dma_start(
                    out=out_d[:, t * NB : (t + 1) * NB], in_=osb[:, t, :]
                ).then_inc(sof, 16)
            sy.wait_ge(sof, 16 * NT)

        @block.scalar
        def _(se):
            se.wait_ge(sg, 1)
            se.activation(dump_a[:], warm[:, 0:16], RELU)  # preload relu table
            se.wait_ge(swr, 32)
            for op in ACT_ORDER:
                if op[0] == "r":
                    _, t, m = op
                    st = t % 2
                    if m == 0 and t >= 2:
                        se.wait_ge(s2, POS_PE[("B", t - 2)])  # h1 set free
                    se.wait_ge(sm, 2 * t + m + 1)
                    se.activation(
                        h1sb[:, st, m, :], ps1[:, st, m, :], RELU,
                        bias=b1v[:, m : m + 1],
                    ).then_inc(sa, 1)
                else:
                    _, t = op
                    st = t % 2
                    se.wait_ge(s2, POS_PE[("C", t)])
                    se.activation(
                        h3sb[:, st, :], ps3[:], RELU, bias=b3v[:]
                    ).then_inc(sa, 1)

        @block.vector
        def _(ve):
            ve.memset(warm[:], 0.125).then_inc(sg, 1)
            ve.wait_ge(sg, 1)
            ve.tensor_scalar(dump_v[:], warm[:, 0:16], 0.0, 0.0, ADD, MAX)
            ve.wait_ge(swr, 32)
            for kind, t in DVE_ORDER:
                st = t % 2
                if kind == "h2":
                    ve.wait_ge(s2, POS_PE[("B", t)])
                    ve.tensor_scalar(
                        h2sb[:, st, :], ps2[:], b2v[:], 0.0, ADD, MAX
                    ).then_inc(sv, 1)
                else:
                    ve.wait_ge(s2, POS_PE[("D", t)])
                    ve.tensor_scalar(
                        osb[:, t, :], ps4[:], b4v[:], None, ADD
                    ).then_inc(sv, 1)

        @block.tensor
        def _(te):
            te.wait_ge(sg, 1)
            for _i in range(N_WARM_MM):
                te.matmul(ps2[0:1, :], warm[:, 0:1], warm[:, 1:513],
                          start=True, stop=True)

            def emit_L1(t):
                st = t % 2
                if t >= 2:
                    te.wait_ge(sa, POS_A[("r", t - 2, 1)])  # ps1 set free
                for m in range(2):
                    for c in range(NKC):
                        if m == 0:
                            for i, (a, _b) in enumerate(X_SPLITS[t]):
                                if a == c:
                                    te.wait_ge(sx[t][i], 16)
                            if t == 0:
                                for i, (a, _b) in enumerate(W1_SPLITS):
                                    if a == c:
                                        te.wait_ge(sw1[i], 16)
                        mm = te.matmul(
                            ps1[:, st, m, :],
                            w1sb[:, c, m * 128 : (m + 1) * 128],
                            xsb[:, t, c, :],
                            start=(c == 0),
                            stop=(c == NKC - 1),
                        )
                        if c == NKC - 1:
                            mm.then_inc(sm, 1)

            for kind, t in PE_ORDER:
                st = t % 2
                if kind == "A":
                    emit_L1(t)
                elif kind == "B":
                    if t == 0:
                        te.wait_ge(swr, 32)
                    te.wait_ge(sa, POS_A[("r", t, 0)])
                    if t >= 1:
                        te.wait_ge(sv, POS_V[("h2", t - 1)])  # ps2 free
                    te.matmul(
                        ps2[:], w2v[:, 0, :], h1sb[:, st, 0, :],
                        start=True, stop=False,
                    )
                    te.wait_ge(sa, POS_A[("r", t, 1)])
                    te.matmul(
                        ps2[:], w2v[:, 1, :], h1sb[:, st, 1, :],
                        start=False, stop=True,
                    ).then_inc(s2, 1)
                elif kind == "C":
                    te.wait_ge(sv, POS_V[("h2", t)])
                    te.matmul(
                        ps3[:], w3v[:], h2sb[:, st, :], start=True, stop=True
                    ).then_inc(s2, 1)
                else:
                    te.wait_ge(sa, POS_A[("h3", t)])
                    if t >= 1:
                        te.wait_ge(sv, POS_V[("out", t - 1)])  # ps4 free
                    te.matmul(
                        ps4[:], w4v[:], h3sb[:, st, :], start=True, stop=True
                    ).then_inc(s2, 1)

    return nc


def _np_dt(dt):
    if dt == BF16:
        return ml_dtypes.bfloat16
    if dt == FP16:
        return np.float16
    return np.float32


def prepare_inputs(x, conv_w, w1, b1, w2, b2, w3, b3, w4, b4,
                   l1_dt=FP16, l234_dt=FP16):
    w1v = np.ascontiguousarray(w1.T).reshape(26, 26, 256)
    w1e = np.zeros((28, 28, 256), dtype=np.float32)
    for di in range(3):
        for dj in range(3):
            w1e[di : di + 26, dj : dj + 26, :] += conv_w[di, dj] * w1v
    w1e = w1e.reshape(784, 256)
    w1t = np.ascontiguousarray(
        w1e.reshape(NKC, KC, 256).transpose(1, 0, 2)
    ).reshape(KC, NKC * 256).astype(_np_dt(l1_dt))

    w2t = np.ascontiguousarray(w2.T).reshape(2, 128, 128).transpose(1, 0, 2)
    wpack = np.zeros((128, 256 + 64 + 10), dtype=np.float32)
    wpack[:, 0:256] = w2t.reshape(128, 256)
    wpack[:, 256:320] = w3.T
    wpack[0:64, 320:330] = w4.T
    wpack = wpack.astype(_np_dt(l234_dt))

    bpack = np.zeros((128, 5), dtype=np.float32)
    bpack[:, 0:2] = b1.reshape(2, 128).T
    bpack[:, 2] = b2
    bpack[0:64, 3] = b3
    bpack[0:10, 4] = b4

    shared = {"w1e": w1t, "wpack": wpack, "bpack": bpack}
    in_maps = []
    for m in range(N_CORES):
        xc = x[m * BC : (m + 1) * BC]
        xt = np.ascontiguousarray(
            xc.reshape(NT, NB, NKC, KC).transpose(0, 3, 2, 1)
        ).reshape(NT, KC, NKC * NB).astype(_np_dt(l1_dt))
        in_maps.append({"xt": xt, **shared})
    return in_maps



_PROGRAM = None


def _get_program():
    global _PROGRAM
    if _PROGRAM is None:
        _PROGRAM = build_program()
    return _PROGRAM


def kernel(x, conv_w, w1, b1, w2, b2, w3, b3, w4, b4):
    from concourse import bass_utils

    args = [x, conv_w, w1, b1, w2, b2, w3, b3, w4, b4]
    x, conv_w, w1, b1, w2, b2, w3, b3, w4, b4 = [
        np.asarray(a, dtype=np.float32) for a in args
    ]
    nc = _get_program()
    in_maps = prepare_inputs(x, conv_w, w1, b1, w2, b2, w3, b3, w4, b4)
    res = bass_utils.run_bass_kernel_spmd(nc, in_maps, list(range(N_CORES)))
    out = np.concatenate(
        [np.ascontiguousarray(res.results[m]["outT"].T) for m in range(N_CORES)],
        axis=0,
    )
    return out.astype(np.float32)
